# revision 22
# baseline (speedup 1.0000x reference)
"""Trainium2 Bass kernel for nn_ABDNet (point-cloud KNN + MLP + transformer encoder).

Sharding: 8 cores = 4 batches x 2 query-halves. Each core handles one batch's
full 2048-point context and 1024 local query points. Cross-core communication:
one AllGather of local K/V per encoder layer within each core pair.

Algorithm notes:
  - First conv + BN + first MLP layer are affine in raw coordinates, so the
    gathered neighbor features reduce to  h1 = relu(u[j] - v[i])  with
    u = M @ xyz_j + d  and  v = Mv @ xyz_i  host-folded.
  - KNN top-32 via VectorE max8/match_replace/max_index.
  - Neighbor gather via indirect DMA from a DRAM u-table.
  - MLP BN folded into weights; mean-pool folded into layer-3 scale.
  - Encoder in T-layout (channels on partitions); softmax without
    max-subtraction (logits are tiny); Z via ones-matmul; 1/Z applied on the
    attention output.
"""

import sys
import numpy as np

for _p in ("/opt/trn_rl_repo", "/opt/pypackages"):
    if _p not in sys.path:
        sys.path.append(_p)

import concourse.bass as bass
import concourse.mybir as mybir
from concourse import bacc
from concourse.bass import IndirectOffsetOnAxis
from concourse.tile import TileContext
from concourse.masks import make_identity

F32 = mybir.dt.float32
F32R = mybir.dt.float32r
U32 = mybir.dt.uint32
AF = mybir.ActivationFunctionType
ALU = mybir.AluOpType
AX = mybir.AxisListType

B = 4
N = 2048          # full points per batch (keys/candidates)
NQ = 1024         # local queries per core
KNN = 32
D = 512
H = 4
DH = 128
DFF = 1024
NCLS = 4
NLAYERS = 3
BN_INV = 1.0 / float(np.sqrt(1.0 + 1e-5))
NEG = -3.0e38
ISQ = 1.0 / float(np.sqrt(128.0))   # 1/sqrt(dh)
LN_EPS = 1e-6

QT = NQ // 128            # 8 query tiles
NCH = N // 128            # 16 candidate/key chunks
NGRP = (NQ * KNN) // 512  # 64 pair groups of 512
GPT = 512 // 128          # 4 gather tiles per group
KVN = H * 128 * NQ        # k part elems in cc buffer


def _bcast_rows(ap, p=128):
    return bass.AP(tensor=ap.tensor, offset=ap.offset, ap=[[0, p]] + list(ap.ap))


def build_nc(num_cores=8):
    nc = bacc.Bacc("TRN2", target_bir_lowering=False, num_devices=num_cores)
    rg = [[2 * i, 2 * i + 1] for i in range(num_cores // 2)]

    # ---------------- DRAM parameters ----------------
    xyz1 = nc.declare_dram_parameter("xyz1", [4, N], F32, isOutput=False)
    xyz1q = nc.declare_dram_parameter("xyz1q", [4, NQ], F32, isOutput=False)
    sqrow_d = nc.declare_dram_parameter("sqrow", [N], F32, isOutput=False)
    maskrow = nc.declare_dram_parameter("maskrow", [N], F32, isOutput=False)
    uw = nc.declare_dram_parameter("uw", [4, 64], F32, isOutput=False)
    vw = nc.declare_dram_parameter("vw", [4, 64], F32, isOutput=False)
    w2t = nc.declare_dram_parameter("w2t", [64, 256], F32, isOutput=False)
    b2d = nc.declare_dram_parameter("b2", [256], F32, isOutput=False)
    w3t = nc.declare_dram_parameter("w3t", [256, 512], F32, isOutput=False)
    b3d = nc.declare_dram_parameter("b3", [512], F32, isOutput=False)
    enc = []
    for l in range(NLAYERS):
        e = {}
        for nm, shp in (("wq", [D, D]), ("wk", [D, D]), ("wv", [D, D]),
                        ("wo", [D, D]), ("w1", [D, DFF]), ("w2f", [DFF, D])):
            e[nm] = nc.declare_dram_parameter(f"{nm}_{l}", shp, F32, isOutput=False)
        for nm in ("bq", "bk", "bv", "bo", "fb2", "g1", "be1", "g2", "be2"):
            e[nm] = nc.declare_dram_parameter(f"{nm}_{l}", [D], F32, isOutput=False)
        e["fb1"] = nc.declare_dram_parameter(f"fb1_{l}", [DFF], F32, isOutput=False)
        enc.append(e)
    clsw = nc.declare_dram_parameter("clsw", [D, NCLS], F32, isOutput=False)
    clsb = nc.declare_dram_parameter("clsb", [NCLS], F32, isOutput=False)
    out_d = nc.declare_dram_parameter("out", [NQ, NCLS], F32, isOutput=True)

    from contextlib import ExitStack
    with TileContext(nc) as tc, ExitStack() as ctx:
        consts = ctx.enter_context(tc.tile_pool(name="consts", bufs=1))
        dram = ctx.enter_context(tc.tile_pool(name="dram", bufs=1, space="DRAM"))

        # ---- persistent SBUF tensors (consts pool, one slot each) ----
        xT = [consts.tile([128, NQ], F32R, name=f"xT{c}") for c in range(4)]
        xtok = [consts.tile([128, D], F32, name=f"xtok{t}") for t in range(QT)]
        x1tok = [consts.tile([128, D], F32, name=f"x1tok{t}") for t in range(QT)]

        ident = consts.tile([128, 128], F32)
        make_identity(nc, ident[:])
        ones_f = consts.tile([128, 128], F32)
        nc.vector.memset(ones_f[:], 1.0)
        ones_c = consts.tile([1, 128], F32R)
        nc.vector.tensor_copy(ones_c[:], ones_f[0:1, :])
        ones_k = consts.tile([128, 1], F32R)
        nc.vector.tensor_copy(ones_k[:], ones_f[:, 0:1])
        eps1 = consts.tile([128, 1], F32)
        nc.vector.memset(eps1[:], LN_EPS)

        xyz1_sb = consts.tile([4, N], F32)
        nc.sync.dma_start(out=xyz1_sb[:], in_=xyz1[:, :])
        xyz1q_sb = consts.tile([4, NQ], F32)
        nc.sync.dma_start(out=xyz1q_sb[:], in_=xyz1q[:, :])
        maskT = consts.tile([128, NCH], F32)
        nc.sync.dma_start(out=maskT[:], in_=maskrow[:].rearrange("(c p) -> p c", p=128))
        sqb = consts.tile([128, N], F32)
        nc.sync.dma_start(out=sqb[:], in_=_bcast_rows(sqrow_d[:]))

        uw_sb = consts.tile([4, 64], F32)
        nc.sync.dma_start(out=uw_sb[:], in_=uw[:, :])
        vw_sb = consts.tile([4, 64], F32)
        nc.sync.dma_start(out=vw_sb[:], in_=vw[:, :])
        w2t_sb = consts.tile([64, 256], F32R)
        nc.sync.dma_start(out=w2t_sb[:], in_=w2t[:, :].bitcast(F32R))
        b2_sb = consts.tile([128, 2], F32)
        nc.sync.dma_start(out=b2_sb[:], in_=b2d[:].rearrange("(c p) -> p c", p=128))
        w3t_sb = [consts.tile([128, 512], F32R, name=f"w3t{k}") for k in range(2)]
        for k in range(2):
            nc.sync.dma_start(out=w3t_sb[k][:],
                              in_=w3t[k * 128:(k + 1) * 128, :].bitcast(F32R))
        b3_sb = consts.tile([128, 4], F32)
        nc.sync.dma_start(out=b3_sb[:], in_=b3d[:].rearrange("(c p) -> p c", p=128))
        clsw_sb = [consts.tile([128, NCLS], F32R, name=f"clsw{k}") for k in range(4)]
        for k in range(4):
            nc.sync.dma_start(out=clsw_sb[k][:],
                              in_=clsw[k * 128:(k + 1) * 128, :].bitcast(F32R))
        clsb_sb = consts.tile([1, NCLS], F32R)
        nc.sync.dma_start(out=clsb_sb[:], in_=clsb[None, :].bitcast(F32R))
        vT = consts.tile([64, NQ], F32)
        gidx = consts.tile([128, NGRP * GPT], U32)

        # ---- DRAM scratch ----
        u_dram = dram.tile([N, 64], F32)
        idx_dram = dram.tile([QT, 128, KNN], U32)
        att_dram = dram.tile([4, 128, NQ], F32)      # attn output, T layout

        # ---- pools ----
        ps_mm = ctx.enter_context(tc.tile_pool(name="ps_mm", bufs=3, space="PSUM"))
        ps_acc = ctx.enter_context(tc.tile_pool(name="ps_acc", bufs=2, space="PSUM"))
        ps_sm = ctx.enter_context(tc.tile_pool(name="ps_sm", bufs=2, space="PSUM"))
        ps_tr = ctx.enter_context(tc.tile_pool(name="ps_tr", bufs=1, space="PSUM"))
        big = ctx.enter_context(tc.tile_pool(name="big", bufs=2))
        work = ctx.enter_context(tc.tile_pool(name="work", bufs=2))
        small = ctx.enter_context(tc.tile_pool(name="small", bufs=2))
        wst = ctx.enter_context(tc.tile_pool(name="wst", bufs=8))
        wsl = ctx.enter_context(tc.tile_pool(name="wsl", bufs=8))    # [128,128] weight slices
        lnb = ctx.enter_context(tc.tile_pool(name="lnb", bufs=1))
        expp = ctx.enter_context(tc.tile_pool(name="expp", bufs=3))
        hTp = ctx.enter_context(tc.tile_pool(name="hTp", bufs=8))
        kvp = ctx.enter_context(tc.tile_pool(name="kvp", bufs=6))    # [128,128] kv slices
        qTp = ctx.enter_context(tc.tile_pool(name="qTp", bufs=1))
        x1Tp = ctx.enter_context(tc.tile_pool(name="x1Tp", bufs=4))

        # =========== STAGE 1: u table, vT ===========
        for c in range(NCH):
            pu = ps_mm.tile([128, 512], F32, tag="psA")
            nc.tensor.matmul(pu[:, :64], lhsT=xyz1_sb[:, c * 128:(c + 1) * 128],
                             rhs=uw_sb[:], start=True, stop=True)
            ut = work.tile([128, 64], F32, tag="ut")
            nc.scalar.activation(ut[:], pu[:, :64], AF.Identity)
            nc.sync.dma_start(out=u_dram[c * 128:(c + 1) * 128, :], in_=ut[:])

        for f in range(2):
            pv = ps_mm.tile([128, 512], F32, tag="psA")
            nc.tensor.matmul(pv[:64, :], lhsT=vw_sb[:],
                             rhs=xyz1q_sb[:, f * 512:(f + 1) * 512],
                             start=True, stop=True)
            nc.scalar.activation(vT[:, f * 512:(f + 1) * 512], pv[:64, :], AF.Identity)

        # =========== STAGE 2: KNN top-32 ===========
        for t in range(QT):
            nd = big.tile([128, N], F32, tag="nd")
            for f in range(4):
                pd = ps_mm.tile([128, 512], F32, tag="psA")
                nc.tensor.matmul(pd[:], lhsT=xyz1q_sb[0:3, t * 128:(t + 1) * 128],
                                 rhs=xyz1_sb[0:3, f * 512:(f + 1) * 512],
                                 start=True, stop=True)
                nc.vector.scalar_tensor_tensor(
                    out=nd[:, f * 512:(f + 1) * 512], in0=pd[:], scalar=2.0,
                    in1=sqb[:, f * 512:(f + 1) * 512],
                    op0=ALU.mult, op1=ALU.subtract)
            V = work.tile([128, 128], F32, tag="topkV")
            for c in range(NCH):
                nc.vector.max(V[:, c * 8:(c + 1) * 8], nd[:, c * 128:(c + 1) * 128])
            m32 = work.tile([128, KNN], F32, tag="m32")
            for r in range(4):
                nc.vector.max(m32[:, r * 8:(r + 1) * 8], V[:])
                if r < 3:
                    nc.vector.match_replace(V[:], in_to_replace=m32[:, r * 8:(r + 1) * 8],
                                            in_values=V[:], imm_value=NEG)
            idx32 = work.tile([128, KNN], U32, tag="idx32")
            for r in range(4):
                nc.vector.max_index(idx32[:, r * 8:(r + 1) * 8],
                                    m32[:, r * 8:(r + 1) * 8], nd[:])
            nc.sync.dma_start(out=idx_dram[t], in_=idx32[:])

        nc.sync.dma_start(out=gidx[:],
                          in_=idx_dram[:].rearrange("a b c -> (a b c)")
                          .rearrange("(c p) -> p c", p=128))

        # =========== STAGE 3: gather + MLP + pool ===========
        for g in range(NGRP):
            gt = ps_tr.tile([64, 512], F32, tag="psT")
            for s in range(GPT):
                G = work.tile([128, 64], F32, tag="G", bufs=4)
                nc.gpsimd.indirect_dma_start(
                    out=G[:], out_offset=None, in_=u_dram[:, :],
                    in_offset=IndirectOffsetOnAxis(
                        ap=gidx[:, g * GPT + s:g * GPT + s + 1], axis=0))
                nc.tensor.transpose(gt[:, s * 128:(s + 1) * 128], G[:], ident[:])
            h1 = work.tile([64, 512], F32R, tag="h1")
            vb = vT[:, g * 16:(g + 1) * 16].to_broadcast([64, 16, KNN])
            nc.vector.tensor_tensor(out=h1[:].rearrange("p (a b) -> p a b", b=KNN),
                                    in0=gt[:].rearrange("p (a b) -> p a b", b=KNN),
                                    in1=vb, op=ALU.subtract)
            nc.vector.tensor_scalar_max(h1[:], h1[:], 0.0)
            h2 = []
            for mc in range(2):
                p2 = ps_mm.tile([128, 512], F32, tag="psA")
                nc.tensor.matmul(p2[:], lhsT=w2t_sb[:, mc * 128:(mc + 1) * 128],
                                 rhs=h1[:], start=True, stop=True)
                h2t = work.tile([128, 512], F32R, tag=f"h2_{mc}", bufs=1)
                nc.scalar.activation(h2t[:], p2[:], AF.Relu, bias=b2_sb[:, mc:mc + 1])
                h2.append(h2t)
            for mc in range(4):
                p3 = ps_acc.tile([128, 512], F32, tag="psAcc")
                for kc in range(2):
                    nc.tensor.matmul(p3[:], lhsT=w3t_sb[kc][:, mc * 128:(mc + 1) * 128],
                                     rhs=h2[kc][:], start=(kc == 0), stop=(kc == 1))
                h3 = work.tile([128, 512], F32, tag="h3")
                nc.scalar.activation(h3[:], p3[:], AF.Relu, bias=b3_sb[:, mc:mc + 1])
                with nc.allow_low_precision(reason="f32r is 4-byte fp32 storage"):
                    nc.vector.reduce_sum(out=xT[mc][:, g * 16:(g + 1) * 16],
                                         in_=h3[:].rearrange("p (a b) -> p a b", b=KNN),
                                         axis=AX.X)

        # =========== helpers ===========
        def load_w_tiles(wd, n_k, width, tagbase):
            tiles = []
            for kc in range(n_k):
                wt = wst.tile([128, width], F32R, tag="wst", name=f"{tagbase}{kc}")
                nc.sync.dma_start(out=wt[:],
                                  in_=wd[kc * 128:(kc + 1) * 128, :].bitcast(F32R))
                tiles.append(wt)
            return tiles

        def load_bias_cols(bd, ncols, tag):
            bt = small.tile([128, ncols], F32, tag=tag, name=tag, bufs=1)
            nc.sync.dma_start(out=bt[:], in_=bd[:].rearrange("(c p) -> p c", p=128))
            return bt

        def load_row(bd, width, tag):
            bt = small.tile([1, width], F32R, tag=tag, name=tag, bufs=1)
            nc.sync.dma_start(out=bt[:], in_=bd[None, :].bitcast(F32R))
            return bt

        def load_bcast(bd, tag):
            bt = lnb.tile([128, D], F32, tag=tag, name=tag)
            nc.sync.dma_start(out=bt[:], in_=_bcast_rows(bd[:]))
            return bt

        def layernorm_tile(x_ps, res_sb, out_tok, gB, bB):
            xs = work.tile([128, D], F32, tag="ln_x")
            nc.vector.tensor_tensor(out=xs[:], in0=x_ps[:], in1=res_sb[:], op=ALU.add)
            st = small.tile([128, 6], F32, tag="ln_st")
            nc.vector.bn_stats(out=st[:], in_=xs[:])
            mv = small.tile([128, 2], F32, tag="ln_mv")
            nc.vector.bn_aggr(out=mv[:], in_=st[:])
            sd = small.tile([128, 1], F32, tag="ln_sd")
            nc.scalar.activation(sd[:], mv[:, 1:2], AF.Sqrt, bias=eps1[:])
            nc.vector.reciprocal(sd[:], sd[:])
            nc.vector.tensor_scalar(out=xs[:], in0=xs[:], scalar1=mv[:, 0:1],
                                    scalar2=sd[:], op0=ALU.subtract, op1=ALU.mult)
            nc.vector.tensor_tensor(out=xs[:], in0=xs[:], in1=gB[:], op=ALU.mult)
            nc.vector.tensor_tensor(out=out_tok[:], in0=xs[:], in1=bB[:], op=ALU.add)

        # x tokens (residual base) before layer 0
        for t in range(QT):
            pt = ps_tr.tile([128, 512], F32, tag="psT")
            for cc in range(4):
                nc.tensor.transpose(pt[:, cc * 128:(cc + 1) * 128],
                                    xT[cc][:, t * 128:(t + 1) * 128].bitcast(F32),
                                    ident[:])
            nc.scalar.activation(xtok[t][:], pt[:], AF.Identity)

        # =========== STAGE 4: encoder layers ===========
        for l in range(NLAYERS):
            e = enc[l]
            # ---- local K (T layout) and V (token layout) -> DRAM -> AllGather ----
            cc_in = dram.tile([KVN + 8 * 128 * D], F32, tag="cc_in", name=f"cc_in{l}")
            k_in = cc_in[:KVN].rearrange("(h p n) -> h p n", h=H, p=128)
            v_in = cc_in[KVN:].rearrange("(t p n) -> t p n", t=8, p=128)

            wk_sb = load_w_tiles(e["wk"], 4, D, "wk")
            bk_sb = load_bias_cols(e["bk"], 4, "bk")
            for h in range(H):
                for f in range(2):
                    pk = ps_mm.tile([128, 512], F32, tag="psA")
                    for kc in range(4):
                        nc.tensor.matmul(pk[:], lhsT=wk_sb[kc][:, h * 128:(h + 1) * 128],
                                         rhs=xT[kc][:, f * 512:(f + 1) * 512],
                                         start=(kc == 0), stop=(kc == 3))
                    ks = work.tile([128, 512], F32, tag="kvs", bufs=4)
                    nc.scalar.activation(ks[:], pk[:], AF.Identity, bias=bk_sb[:, h:h + 1])
                    nc.sync.dma_start(out=k_in[h, :, f * 512:(f + 1) * 512], in_=ks[:])
            wv_sb = load_w_tiles(e["wv"], 4, D, "wv")
            bv_row = load_row(e["bv"], D, "bv")
            for t in range(QT):
                pv = ps_mm.tile([128, 512], F32, tag="psA")
                for kc in range(4):
                    nc.tensor.matmul(pv[:], lhsT=xT[kc][:, t * 128:(t + 1) * 128],
                                     rhs=wv_sb[kc][:], start=(kc == 0), stop=False)
                nc.tensor.matmul(pv[:], lhsT=ones_c[:], rhs=bv_row[:],
                                 start=False, stop=True)
                vs = work.tile([128, 512], F32, tag="kvs", bufs=4)
                nc.scalar.activation(vs[:], pv[:], AF.Identity)
                nc.sync.dma_start(out=v_in[t], in_=vs[:])

            cc_out = dram.tile([2, KVN + 8 * 128 * D], F32, tag="cc_out",
                               name=f"cc_out{l}")
            nc.gpsimd.collective_compute(
                "AllGather", ALU.bypass, replica_groups=rg,
                ins=[cc_in[:].opt()], outs=[cc_out[:].opt()])
            k_out = [cc_out[hh, :KVN].rearrange("(h p n) -> h p n", h=H, p=128)
                     for hh in range(2)]
            v_out = [cc_out[hh, KVN:].rearrange("(t p n) -> t p n", t=8, p=128)
                     for hh in range(2)]

            # ---- attention (per head: Q proj, then per key-chunk) ----
            wq_sb = load_w_tiles(e["wq"], 4, D, "wq")
            bq_sb = load_bias_cols(e["bq"], 4, "bq")
            for h in range(H):
                qT = qTp.tile([128, NQ], F32R, tag="qT")
                for f in range(2):
                    pq = ps_mm.tile([128, 512], F32, tag="psA")
                    for kc in range(4):
                        nc.tensor.matmul(pq[:], lhsT=wq_sb[kc][:, h * 128:(h + 1) * 128],
                                         rhs=xT[kc][:, f * 512:(f + 1) * 512],
                                         start=(kc == 0), stop=(kc == 3))
                    nc.scalar.activation(qT[:, f * 512:(f + 1) * 512], pq[:],
                                         AF.Identity, bias=bq_sb[:, h:h + 1])
                pa = [ps_acc.tile([128, 512], F32, tag="psAcc", name=f"pa{i}")
                      for i in range(2)]
                pz = [ps_sm.tile([1, 512], F32, tag="psZ", name=f"pz{i}")
                      for i in range(2)]
                for kc in range(NCH):
                    hh, tl = kc // 8, kc % 8
                    kt = kvp.tile([128, 128], F32R, tag="kv")
                    nc.sync.dma_start(out=kt[:],
                                      in_=k_out[hh][h, :, tl * 128:(tl + 1) * 128].bitcast(F32R))
                    vt = kvp.tile([128, 128], F32R, tag="kv")
                    nc.sync.dma_start(out=vt[:],
                                      in_=v_out[hh][tl][:, h * 128:(h + 1) * 128].bitcast(F32R))
                    for qc in range(2):
                        pl = ps_mm.tile([128, 512], F32, tag="psA")
                        nc.tensor.matmul(pl[:], lhsT=kt[:],
                                         rhs=qT[:, qc * 512:(qc + 1) * 512],
                                         start=True, stop=True)
                        ex = expp.tile([128, 512], F32R, tag="exp")
                        nc.scalar.activation(ex[:], pl[:], AF.Exp,
                                             bias=maskT[:, kc:kc + 1], scale=ISQ)
                        nc.tensor.matmul(pa[qc][:], lhsT=vt[:], rhs=ex[:],
                                         start=(kc == 0), stop=(kc == NCH - 1),
                                         skip_group_check=True)
                        nc.tensor.matmul(pz[qc][:], lhsT=ones_k[:], rhs=ex[:],
                                         start=(kc == 0), stop=(kc == NCH - 1),
                                         skip_group_check=True)
                for qc in range(2):
                    zrow = small.tile([1, 512], F32R, tag="zrow")
                    nc.scalar.activation(zrow[:], pz[qc][:], AF.Identity)
                    pzb = ps_mm.tile([128, 512], F32, tag="psA")
                    nc.tensor.matmul(pzb[:], lhsT=ones_c[:], rhs=zrow[:],
                                     start=True, stop=True)
                    zrb = work.tile([128, 512], F32, tag="zrb")
                    nc.vector.reciprocal(zrb[:], pzb[:])
                    ao = work.tile([128, 512], F32, tag="ao")
                    nc.vector.tensor_tensor(out=ao[:], in0=pa[qc][:], in1=zrb[:],
                                            op=ALU.mult)
                    nc.sync.dma_start(out=att_dram[h][:, qc * 512:(qc + 1) * 512],
                                      in_=ao[:])

            # ---- O-proj + residual + LN1 ----
            wo_sb = load_w_tiles(e["wo"], 4, D, "wo")
            bo_row = load_row(e["bo"], D, "bo")
            g1B = load_bcast(e["g1"], "g1B")
            b1B = load_bcast(e["be1"], "b1B")
            for tc in range(QT):
                po = ps_acc.tile([128, 512], F32, tag="psAcc")
                for cc in range(4):
                    at = kvp.tile([128, 128], F32R, tag="kv")
                    nc.sync.dma_start(out=at[:],
                                      in_=att_dram[cc][:, tc * 128:(tc + 1) * 128].bitcast(F32R))
                    nc.tensor.matmul(po[:], lhsT=at[:], rhs=wo_sb[cc][:],
                                     start=(cc == 0), stop=False)
                nc.tensor.matmul(po[:], lhsT=ones_c[:], rhs=bo_row[:],
                                 start=False, stop=True)
                layernorm_tile(po, xtok[tc], x1tok[tc], g1B, b1B)

            # ---- FFN (token groups of 512) ----
            fb1_sb = load_bias_cols(e["fb1"], 8, "fb1")
            w2f_sb = load_w_tiles(e["w2f"], 8, D, "w2f")
            fb2_row = load_row(e["fb2"], D, "fb2")
            g2B = load_bcast(e["g2"], "g2B")
            b2B = load_bcast(e["be2"], "b2B")
            for fg in range(2):
                # x1T for this token group via PE transpose
                x1Tg = [x1Tp.tile([128, 512], F32R, tag="x1Tg", name=f"x1Tg{cc}")
                        for cc in range(4)]
                for ti in range(4):
                    tcg = fg * 4 + ti
                    pt = ps_tr.tile([128, 512], F32, tag="psT")
                    for cc in range(4):
                        nc.tensor.transpose(pt[:, cc * 128:(cc + 1) * 128],
                                            x1tok[tcg][:, cc * 128:(cc + 1) * 128],
                                            ident[:])
                    for cc in range(4):
                        if cc % 2 == 0:
                            nc.scalar.activation(x1Tg[cc][:, ti * 128:(ti + 1) * 128],
                                                 pt[:, cc * 128:(cc + 1) * 128],
                                                 AF.Identity)
                        else:
                            nc.vector.tensor_copy(x1Tg[cc][:, ti * 128:(ti + 1) * 128],
                                                  pt[:, cc * 128:(cc + 1) * 128])
                hT = []
                for mc in range(8):
                    ph = ps_mm.tile([128, 512], F32, tag="psA")
                    for kc in range(4):
                        wsl_t = wsl.tile([128, 128], F32R, tag="wsl")
                        nc.sync.dma_start(out=wsl_t[:],
                                          in_=e["w1"][kc * 128:(kc + 1) * 128,
                                                      mc * 128:(mc + 1) * 128].bitcast(F32R))
                        nc.tensor.matmul(ph[:], lhsT=wsl_t[:], rhs=x1Tg[kc][:],
                                         start=(kc == 0), stop=(kc == 3),
                                         skip_group_check=True)
                    ht = hTp.tile([128, 512], F32R, tag="hT")
                    nc.scalar.activation(ht[:], ph[:], AF.Relu, bias=fb1_sb[:, mc:mc + 1])
                    hT.append(ht)
                for ti in range(4):
                    tcg = fg * 4 + ti
                    pf = ps_acc.tile([128, 512], F32, tag="psAcc")
                    for kc in range(8):
                        nc.tensor.matmul(pf[:], lhsT=hT[kc][:, ti * 128:(ti + 1) * 128],
                                         rhs=w2f_sb[kc][:], start=(kc == 0), stop=False)
                    nc.tensor.matmul(pf[:], lhsT=ones_c[:], rhs=fb2_row[:],
                                     start=False, stop=True)
                    layernorm_tile(pf, x1tok[tcg], xtok[tcg], g2B, b2B)

            # ---- x2 -> xT (T layout) for next layer / classifier ----
            for t in range(QT):
                pt = ps_tr.tile([128, 512], F32, tag="psT")
                for cc in range(4):
                    nc.tensor.transpose(pt[:, cc * 128:(cc + 1) * 128],
                                        xtok[t][:, cc * 128:(cc + 1) * 128], ident[:])
                for cc in range(4):
                    if cc % 2 == 0:
                        nc.scalar.activation(xT[cc][:, t * 128:(t + 1) * 128],
                                             pt[:, cc * 128:(cc + 1) * 128], AF.Identity)
                    else:
                        nc.vector.tensor_copy(xT[cc][:, t * 128:(t + 1) * 128],
                                              pt[:, cc * 128:(cc + 1) * 128])

        # =========== STAGE 5: classifier + log_softmax ===========
        for tc in range(QT):
            pc_ps = ps_mm.tile([128, 512], F32, tag="psA")
            for cc in range(4):
                nc.tensor.matmul(pc_ps[:, :NCLS], lhsT=xT[cc][:, tc * 128:(tc + 1) * 128],
                                 rhs=clsw_sb[cc][:], start=(cc == 0), stop=False)
            nc.tensor.matmul(pc_ps[:, :NCLS], lhsT=ones_c[:], rhs=clsb_sb[:],
                             start=False, stop=True)
            mx = small.tile([128, 1], F32, tag="mx")
            nc.vector.reduce_max(out=mx[:], in_=pc_ps[:, :NCLS], axis=AX.X)
            mxn = small.tile([128, 1], F32, tag="mxn")
            nc.vector.tensor_scalar_mul(mxn[:], mx[:], -1.0)
            et = small.tile([128, NCLS], F32, tag="et")
            zs = small.tile([128, 1], F32, tag="zs")
            nc.scalar.activation(et[:], pc_ps[:, :NCLS], AF.Exp, bias=mxn[:],
                                 accum_out=zs[:])
            lnz = small.tile([128, 1], F32, tag="lnz")
            nc.scalar.activation(lnz[:], zs[:], AF.Ln)
            osb = small.tile([128, NCLS], F32, tag="osb")
            nc.vector.tensor_scalar(out=osb[:], in0=pc_ps[:, :NCLS], scalar1=mx[:],
                                    scalar2=lnz[:], op0=ALU.subtract, op1=ALU.subtract)
            nc.sync.dma_start(out=out_d[tc * 128:(tc + 1) * 128, :], in_=osb[:])

    nc.compile()
    return nc


# ======================= host side =======================

def _np(x):
    return np.asarray(x, dtype=np.float32)


def fold_params(params):
    fh = params["first_hd"]
    w0, b0, g0, be0 = _np(fh["w"]), _np(fh["b"]), _np(fh["g"]), _np(fh["be"])
    s0 = BN_INV * g0
    A = s0[:, None] * w0
    cvec = s0 * b0 + be0
    m1 = params["mlp"][0]
    w1, b1, g1, be1 = _np(m1["w"]), _np(m1["b"]), _np(m1["g"]), _np(m1["be"])
    W1a, W1b = w1[:, :3], w1[:, 3:]
    s1 = BN_INV * g1
    M = s1[:, None] * (W1a + W1b @ A)
    dvec = s1 * (W1b @ cvec + b1) + be1
    Mv = s1[:, None] * W1a
    uw = np.concatenate([M.T, dvec[None, :]], axis=0).astype(np.float32)
    vw = np.concatenate([Mv.T, np.zeros((1, 64), np.float32)], axis=0)

    m2 = params["mlp"][1]
    s2 = BN_INV * _np(m2["g"])
    W2p = s2[:, None] * _np(m2["w"])
    b2 = s2 * _np(m2["b"]) + _np(m2["be"])
    m3 = params["mlp"][2]
    s3 = BN_INV * _np(m3["g"])
    W3p = (s3[:, None] * _np(m3["w"])) / float(KNN)
    b3 = (s3 * _np(m3["b"]) + _np(m3["be"])) / float(KNN)

    weights = {
        "uw": uw, "vw": vw,
        "w2t": np.ascontiguousarray(W2p.T), "b2": b2,
        "w3t": np.ascontiguousarray(W3p.T), "b3": b3,
        "clsw": np.ascontiguousarray(_np(params["cls"]["w"]).T),
        "clsb": _np(params["cls"]["b"]),
    }
    for l, e in enumerate(params["enc"]):
        weights[f"wq_{l}"] = _np(e["wq"])
        weights[f"bq_{l}"] = _np(e["bq"])
        weights[f"wk_{l}"] = _np(e["wk"])
        weights[f"bk_{l}"] = _np(e["bk"])
        weights[f"wv_{l}"] = _np(e["wv"])
        weights[f"bv_{l}"] = _np(e["bv"])
        weights[f"wo_{l}"] = _np(e["wo"])
        weights[f"bo_{l}"] = _np(e["bo"])
        weights[f"w1_{l}"] = _np(e["w1"])
        weights[f"fb1_{l}"] = _np(e["b1"])
        weights[f"w2f_{l}"] = _np(e["w2"])
        weights[f"fb2_{l}"] = _np(e["b2"])
        weights[f"g1_{l}"] = _np(e["ln1g"])
        weights[f"be1_{l}"] = _np(e["ln1b"])
        weights[f"g2_{l}"] = _np(e["ln2g"])
        weights[f"be2_{l}"] = _np(e["ln2b"])
    return weights


def make_in_maps(pc, mask, params, num_cores=8):
    weights = fold_params(params)
    pc = _np(pc)
    mask = _np(mask)
    in_maps = []
    for core in range(num_cores):
        b, half = core // 2, core % 2
        xyz1 = np.concatenate([pc[b], np.ones((1, N), np.float32)], axis=0)
        m = dict(weights)
        m["xyz1"] = np.ascontiguousarray(xyz1)
        m["xyz1q"] = np.ascontiguousarray(xyz1[:, half * NQ:(half + 1) * NQ])
        m["sqrow"] = np.ascontiguousarray((pc[b] * pc[b]).sum(axis=0))
        m["maskrow"] = np.ascontiguousarray(mask[b, 0, 0, :])
        in_maps.append(m)
    return in_maps


_CACHED_NC = {}


def _get_nc(num_cores=8):
    if num_cores not in _CACHED_NC:
        _CACHED_NC[num_cores] = build_nc(num_cores)
    return _CACHED_NC[num_cores]


def run(pc, mask, params, num_cores=8, trace=False, **kw):
    import os
    from concourse.bass_utils import run_bass_kernel_spmd
    nc = _get_nc(num_cores)
    in_maps = make_in_maps(pc, mask, params, num_cores)
    env_backup = None
    if not trace and "BASS_NEVER_TRACE" not in os.environ:
        # Some environments set BASS_TRACE globally, but the axon ntff hook
        # is not installable everywhere; force it off for plain runs.
        env_backup = os.environ.get("BASS_NEVER_TRACE")
        os.environ["BASS_NEVER_TRACE"] = "1"
    try:
        res = run_bass_kernel_spmd(nc, in_maps, core_ids=list(range(num_cores)),
                                   trace=trace, **kw)
    finally:
        if env_backup is None and not trace:
            os.environ.pop("BASS_NEVER_TRACE", None)
    out = np.zeros((B, N, NCLS), np.float32)
    for core in range(num_cores):
        b, half = core // 2, core % 2
        out[b, half * NQ:(half + 1) * NQ, :] = res.results[core]["out"]
    return out, res


def kernel(pc, mask, params):
    out, _ = run(pc, mask, params, num_cores=8, trace=False)
    return out
